# revision 42
# baseline (speedup 1.0000x reference)
"""Single-head attention (B=4, S=4096, E=2048, d=128) on 8 trn2 cores.

Sharding: core c handles (batch b = c//2, seq half h = c%2). Each core
projects q/k/v for its own 2048-row half; the pair (2b, 2b+1) exchanges
K|V with one 2-core AllGather (~30us doorbell-to-data latency mostly
independent of payload, so a single early doorbell). Softmax over keys
is permutation-invariant, so per-core key order (own-first) is harmless.

Bias algebra: k-bias shifts every key score of a query by a per-query
constant -> softmax-invariant -> dropped. v-bias adds bv to the output
post-normalization -> added on the host. Only the q-bias is applied on
device (folded into the q PSUM evacuation on the DVE).

Measured constraints that shape the schedule: the x load is DMA-bound
(~10MB at ~330GB/s aggregate over 3 queues = all data in at ~40us, and
every projection needs all of it), each engine queue is strict FIFO
(emission order = run order), the PE runs N=512 matmuls at ~216ns
(2.4GHz; the chip sometimes throttles to 2.0GHz run-to-run), and ACT's
~73us exp stream is the attention-phase pace-setter, gated on q (pass
A) and on peer K (pass B). Schedule:
  junk warmup MMs (HAM un-throttle) | x on 3 queues as 32 half-tiles
  load phase: k-FULL + v-FULL matmuls chase the arriving tiles, so the
    K|V AllGather doorbell rings the moment x lands (+evac)
  q in QB-quarters with scores+exp-A chasing each (first exp ~8us
    earlier than half-granularity); own-v PE-transposes ride the
    qb0 scores stream so their DVE copies sit early in the DVE FIFO
  peer V lands pre-transposed via DMA-xbar transposes from cc_out
  PV-A(qb) interleaved per-k-pair with scores-B(qb-1); PV-B0 under
    scores-B3; only PV-B3 + ~2.3us of tree trail the final exp
Denominators: tree levels LA/LB ride the exp stream on the DVE; the
deferred M/N/C3 never stall the PE; the final 128-partition reduction
of the C3 partials happens on the host from a [8, 128, 512] output.
"""

import numpy as np
import ml_dtypes

import concourse.tile as tile
from concourse.masks import make_identity
from concourse import bacc, mybir
from concourse.bass_utils import run_bass_kernel_spmd

N_CORES = 8
B, S, E, D = 4, 4096, 2048, 128
HALF = S // 2  # queries / own keys per core
QB = 512  # query block (PSUM bank width in fp32)
NE = E // 128  # 16 e-chunks
NQB = HALF // QB  # 4 query blocks
SCALE = 1.0 / float(np.sqrt(D))

BF16 = mybir.dt.bfloat16
F32 = mybir.dt.float32
AF = mybir.ActivationFunctionType
GROUPS = [[2 * i, 2 * i + 1] for i in range(N_CORES // 2)]

_CACHE = {}


def _build():
    nc = bacc.Bacc(
        trn_type="TRN2", target_bir_lowering=False, debug=False, num_devices=N_CORES
    )

    x_d = nc.dram_tensor("xt", [E, HALF], BF16, kind="ExternalInput").ap()
    # w packed cg-major: [128, cg(3) * e(16) * 128], cg order (k, q, v)
    w_d = nc.dram_tensor("w", [128, 3 * NE * 128], BF16, kind="ExternalInput").ap()
    bias_d = nc.dram_tensor("bias_q", [D, 1], F32, kind="ExternalInput").ap()
    peer_d = nc.dram_tensor("peer", [1, 1], mybir.dt.uint32, kind="ExternalInput").ap()
    out_d = nc.dram_tensor("out_t", [D, HALF], F32, kind="ExternalOutput").ap()
    # C3-level exp partial sums [pass*4+qb, key-in-chunk, q]; host reduces
    sums_d = nc.dram_tensor("sums_c3", [8, 128, QB], BF16, kind="ExternalOutput").ap()

    with tile.TileContext(nc) as tc:
        with (
            tc.tile_pool(name="xt", bufs=16) as xt_pool,
            tc.tile_pool(name="wsb", bufs=1) as w_pool,
            tc.tile_pool(name="persist", bufs=1) as persist,
            tc.tile_pool(name="exp", bufs=5) as exp_pool,
            tc.tile_pool(name="comb", bufs=2) as comb_pool,
            tc.tile_pool(name="dram", bufs=1, space="DRAM") as dram_pool,
            tc.tile_pool(name="ps", bufs=4, space="PSUM") as ps_pool,
        ):
            # ---- warmup fodder first so the PE can start immediately ----
            junk = persist.tile([128, QB], BF16, tag="junk")
            nc.gpsimd.memset(junk[:], 0.0)
            ps_warm = ps_pool.tile([128, 2 * QB], F32, tag="ps")
            for _ in range(6):
                nc.tensor.matmul(
                    ps_warm[:, 0:QB], lhsT=junk[:, 0:128], rhs=junk[:],
                    start=True, stop=True,
                )
            ident = persist.tile([128, 128], BF16, tag="ident")
            make_identity(nc, ident[:])
            bias_sb = persist.tile([D, 1], F32, tag="bias")
            nc.gpsimd.dma_start(bias_sb[:], bias_d[:])

            # peer slot register (host supplies 1 on even cores, 0 on odd)
            peer_reg = nc.sync.alloc_register("peer_slot")
            nc.sync.reg_load(peer_reg, peer_d[0:1, 0:1])
            peer_val = nc.sync.snap(peer_reg, donate=True, min_val=0, max_val=1)

            # ---- weight / x loads; w interleaved by first use ----
            w_sb = w_pool.tile([128, 3 * NE * 128], BF16, tag="w")
            WG = NE * 128  # one cg = 2048 cols

            def w_ap(cg, e):
                return w_sb[:, cg * WG + e * 128 : cg * WG + (e + 1) * 128]

            def w_load(cg, half, eng):
                lo = cg * WG + half * WG // 2
                eng.dma_start(w_sb[:, lo : lo + WG // 2], w_d[:, lo : lo + WG // 2])

            # k weights on the two fast queues; v/q weights head the scalar
            # queue (idle early); scalar x tiles only from e4 on (its first
            # transfers land late behind the engine preamble)
            w_load(0, 0, nc.sync)
            w_load(0, 1, nc.gpsimd)
            w_load(2, 0, nc.scalar)
            w_load(2, 1, nc.scalar)
            w_load(1, 0, nc.scalar)
            w_load(1, 1, nc.scalar)
            xt = {}
            rr = (nc.sync, nc.gpsimd, nc.scalar)
            for e in range(NE):
                t = xt_pool.tile([128, HALF], BF16, tag="xt")
                xt[e] = t
                for h in range(2):  # half-tiles: finer arrival granularity
                    u = 2 * e + h
                    eng = rr[u % 3] if u >= 4 else (nc.sync, nc.gpsimd)[u % 2]
                    eng.dma_start(
                        t[:, h * 2 * QB : (h + 1) * 2 * QB],
                        x_d[e * 128 : (e + 1) * 128, h * 2 * QB : (h + 1) * 2 * QB],
                    )

            # ---- persistent activations ----
            qT = persist.tile([D, HALF], BF16, tag="qT")
            k_sb = persist.tile([D, S], BF16, tag="k_sb")  # [own kT | peer kT]
            vT_own = persist.tile([D, HALF], BF16, tag="vT_own")
            v_sb = persist.tile([128, (S // 128) * D], BF16, tag="v_sb")
            o_stage = persist.tile([D, HALF], F32, tag="o_stage")

            def k_ap(c):  # kT chunk c (d on partitions); own 0-15, peer 16-31
                return k_sb[:, c * 128 : (c + 1) * 128]

            # ---- load phase: k-FULL + v-FULL chase the arriving x tiles
            # (both gate the combined K|V AllGather; v lags 2 e-chunks so
            # the wv DMA arrives) ----
            ps_ka = ps_pool.tile([128, 2 * QB], F32, tag="ps")
            ps_kb = ps_pool.tile([128, 2 * QB], F32, tag="ps")
            ps_va = ps_pool.tile([128, 2 * QB], F32, tag="ps")
            ps_vb = ps_pool.tile([128, 2 * QB], F32, tag="ps")
            pk = [ps_ka, ps_kb]
            pv = [ps_va, ps_vb]
            LAG = 4  # half-tile units
            for step in range(2 * NE + LAG):
                for cg, pd, u in ((0, pk, step), (2, pv, step - LAG)):
                    if 0 <= u < 2 * NE:
                        e, h = u // 2, u % 2
                        for blk in (2 * h, 2 * h + 1):
                            nc.tensor.matmul(
                                pd[blk // 2][:, (blk % 2) * QB : (blk % 2 + 1) * QB],
                                lhsT=w_ap(cg, e),
                                rhs=xt[e][:, blk * QB : (blk + 1) * QB],
                                start=(e == 0),
                                stop=(e == NE - 1),
                            )
                if step < LAG:  # keep PE busy/warm while DMA ramps
                    for _ in range(2):
                        nc.tensor.matmul(
                            ps_warm[:, 0:QB], lhsT=junk[:, 0:128], rhs=junk[:],
                            start=True, stop=True,
                        )
            for i in range(2):  # ACT is idle until the first exp
                nc.scalar.copy(k_sb[:, i * 2 * QB : (i + 1) * 2 * QB], pk[i][:])
            for i in range(2):
                nc.vector.tensor_copy(
                    vT_own[:, i * 2 * QB : (i + 1) * 2 * QB], pv[i][:]
                )

            # ---- combined K|V exchange (1MB AllGather, one doorbell) ----
            cc_in = dram_pool.tile([D, S], BF16, tag="cc_in")
            cc_out = dram_pool.tile([2, D, S], BF16, tag="cc_out")
            nc.sync.dma_start(cc_in[:, HALF:S], vT_own[:])
            nc.gpsimd.dma_start(cc_in[:, 0:HALF], k_sb[:, 0:HALF])
            nc.gpsimd.collective_compute(
                "AllGather",
                mybir.AluOpType.bypass,
                replica_groups=GROUPS,
                ins=[cc_in.opt()],
                outs=[cc_out.opt()],
            )
            nc.sync.dma_start(k_sb[:, HALF:S], cc_out[peer_val][:, 0:HALF])
            for c in range(16):
                nc.sync.dma_start_transpose(
                    v_sb[:, (16 + c) * D : (17 + c) * D],
                    cc_out[peer_val][:, HALF + c * 128 : HALF + (c + 1) * 128],
                )

            # ---- attention machinery ----
            exp_regions = {}
            comb_regions = {}
            W4, W2, W1 = 4 * QB, 2 * QB, QB

            def scores_kp(qb, p, kp):
                """Scores + exp for one k-pair; tree levels LA/LB ride the
                exp stream (kp3/kp7) so only ~2.3us of tree trails a pass."""
                ex = exp_regions[(qb, p)]
                q_ap = qT[:, qb * QB : (qb + 1) * QB]
                ps = ps_pool.tile([128, 2 * QB], F32, tag="ps")
                for half in range(2):
                    nc.tensor.matmul(
                        ps[:, half * QB : (half + 1) * QB],
                        lhsT=k_ap(16 * p + 2 * kp + half),
                        rhs=q_ap,
                        start=True,
                        stop=True,
                    )
                nc.scalar.activation(
                    ex[:, kp * 2 * QB : (kp + 1) * 2 * QB], ps[:], AF.Exp,
                    scale=SCALE,
                )
                if kp == 3:
                    cb_reg = comb_pool.tile([128, 8 * QB], BF16, tag="comb")
                    comb_regions[(qb, p)] = cb_reg
                    cb = cb_reg
                    nc.vector.tensor_add(
                        cb[:, 0:W4], ex[:, 0:W4], ex[:, W4 : 2 * W4]
                    )
                if kp == 7:
                    cb = comb_regions[(qb, p)]
                    nc.vector.tensor_add(
                        cb[:, W4 : 2 * W4], ex[:, 2 * W4 : 3 * W4],
                        ex[:, 3 * W4 : 4 * W4],
                    )

            def scores_exp(qb, p):
                ex_reg = exp_pool.tile([128, 16 * QB], BF16, tag="exp")
                exp_regions[(qb, p)] = ex_reg
                for kp in range(8):
                    scores_kp(qb, p, kp)

            def pv_kp(qb, p, ps_o, kp):
                ex = exp_regions[(qb, p)]
                for half in range(2):
                    c = 16 * p + 2 * kp + half
                    off = kp * 2 * QB + half * QB
                    nc.tensor.matmul(
                        ps_o[:, 0:QB],
                        lhsT=v_sb[:, c * D : (c + 1) * D],
                        rhs=ex[:, off : off + QB],
                        start=(kp == 0 and half == 0),
                        stop=(kp == 7 and half == 1),
                    )

            def finish_evac(qb, p, ps_o):
                """Output evacuation right after the PV accumulation."""
                exp_regions.pop((qb, p))
                o_sl = o_stage[:, qb * QB : (qb + 1) * QB]
                if p == 0:
                    nc.vector.tensor_copy(o_sl, ps_o[:, 0:QB])
                else:
                    nc.vector.tensor_add(o_sl, o_sl, ps_o[:, 0:QB])
                    nc.sync.dma_start(out_d[:, qb * QB : (qb + 1) * QB], o_sl)

            def finish_sums(qb, p):
                """Deferred M/N/C3 tree; the 128-partition reduction of C3
                happens on the host (no PE/PSUM on this path)."""
                cb = comb_regions.pop((qb, p))
                la, lb = cb[:, 0:W4], cb[:, W4 : 2 * W4]
                m = la  # in-place exact-overlap; N/C3 reuse LB (dead after M)
                n = lb[:, 0:W2]
                c3 = lb[:, W2 : W2 + W1]
                nc.vector.tensor_add(m, la, lb)
                nc.vector.tensor_add(n, m[:, 0:W2], m[:, W2 : 2 * W2])
                nc.vector.tensor_add(c3, n[:, 0:W1], n[:, W1 : 2 * W1])
                nc.sync.dma_start(sums_d[4 * p + qb], c3)

            def pv_block(qb, p):
                ps_o = ps_pool.tile([128, 2 * QB], F32, tag="ps")
                for kp in range(8):
                    pv_kp(qb, p, ps_o, kp)
                finish_evac(qb, p, ps_o)

            def project_q(qb):
                """One QB-quarter of the q projection: the first exp only
                needs qb0, so quarters start the ACT stream ~8us earlier."""
                ps = ps_pool.tile([128, 2 * QB], F32, tag="ps")
                for e in range(NE):
                    nc.tensor.matmul(
                        ps[:, 0:QB],
                        lhsT=w_ap(1, e),
                        rhs=xt[e][:, qb * QB : (qb + 1) * QB],
                        start=(e == 0),
                        stop=(e == NE - 1),
                    )
                nc.vector.tensor_scalar_add(
                    qT[:, qb * QB : (qb + 1) * QB], ps[:, 0:QB], bias_sb[:]
                )

            # ---- pass A emission: q halves, exp stream chasing each ----
            def v_transpose(c):
                # bf16 [128, 2048] = same 4KB/partition as the fp32 tiles
                ps_t = ps_pool.tile([128, 4 * QB], BF16, tag="ps")
                nc.tensor.transpose(
                    ps_t[:, 0:128], vT_own[:, c * 128 : (c + 1) * 128], ident[:]
                )
                nc.vector.tensor_copy(v_sb[:, c * D : (c + 1) * D], ps_t[:, 0:128])

            def scores_exp_with_transposes(qb, p):
                """qb0/qb1-A variant: the 16 own-v PE transposes and their
                DVE copies ride the scores/exp stream (2 per k-pair) so the
                copies sit early in the DVE FIFO."""
                ex_reg = exp_pool.tile([128, 16 * QB], BF16, tag="exp")
                exp_regions[(qb, p)] = ex_reg
                for kp in range(8):
                    scores_kp(qb, p, kp)
                    if qb == 0:
                        v_transpose(2 * kp)
                        v_transpose(2 * kp + 1)

            project_q(0)
            scores_exp_with_transposes(0, 0)
            project_q(1)
            scores_exp(1, 0)
            project_q(2)
            scores_exp(2, 0)
            project_q(3)

            # ---- stagger: PV-A(qb) interleaved per-kp with scores-B(qb-1)
            # so ACT streams exp continuously into pass B; sums deferred ----
            pv_block(0, 0)
            scores_exp(3, 0)  # uses the region freed by pv A0
            finish_sums(0, 0)
            for qb_pv, qb_sc in ((1, 0), (2, 1), (3, 2)):
                ex_reg = exp_pool.tile([128, 16 * QB], BF16, tag="exp")
                exp_regions[(qb_sc, 1)] = ex_reg
                ps_o = ps_pool.tile([128, 2 * QB], F32, tag="ps")
                for kp in range(8):
                    pv_kp(qb_pv, 0, ps_o, kp)
                    scores_kp(qb_sc, 1, kp)
                finish_evac(qb_pv, 0, ps_o)
                finish_sums(qb_pv, 0)
            # PV-B0 rides under scores/exp-B3; B1/B2 run during exp-B3;
            # only PV-B3 + ~2.3us of tree trail the final exp
            ex_reg = exp_pool.tile([128, 16 * QB], BF16, tag="exp")
            exp_regions[(3, 1)] = ex_reg
            ps_o0 = ps_pool.tile([128, 2 * QB], F32, tag="ps")
            for kp in range(8):
                pv_kp(0, 1, ps_o0, kp)
                scores_kp(3, 1, kp)
            finish_evac(0, 1, ps_o0)
            for qb in range(1, NQB):
                pv_block(qb, 1)
                finish_sums(qb - 1, 1)
            finish_sums(3, 1)

    nc.compile()
    return nc


def _prep_inputs(x, W, b):
    """Host-side sharding prep: cast bf16, transpose to xT, pack w cg-major."""
    b_f = np.asarray(b, dtype=np.float32)
    bias_q = np.ascontiguousarray(b_f[0:D].reshape(D, 1))  # q bias column
    # W [E, 3D] -> [128p, cg(3), e(16), 128] with cg order (k, q, v)
    w4 = np.asarray(W).astype(ml_dtypes.bfloat16).reshape(NE, 128, 3, D)
    w_bf = np.ascontiguousarray(
        w4.transpose(1, 2, 0, 3)[:, [1, 0, 2], :, :].reshape(128, 3 * NE * D)
    )
    in_maps = []
    for bb in range(B):
        xt_full = np.ascontiguousarray(
            np.asarray(x[bb]).astype(ml_dtypes.bfloat16).T
        )  # [E, S]
        for h in range(2):
            xc = np.ascontiguousarray(xt_full[:, h * HALF : (h + 1) * HALF])
            peer = np.array([[1 - h]], dtype=np.uint32)
            in_maps.append(
                {"xt": xc, "w": w_bf, "bias_q": bias_q, "peer": peer}
            )
    return in_maps


def _run(in_maps, trace=False, trace_kwargs=None):
    if "nc" not in _CACHE:
        _CACHE["nc"] = _build()
    return run_bass_kernel_spmd(
        _CACHE["nc"],
        in_maps,
        list(range(N_CORES)),
        trace=trace,
        **(trace_kwargs or {}),
    )


def kernel(x, W, b):
    in_maps = _prep_inputs(x, W, b)
    res = None
    for attempt in range(3):
        try:
            res = _run(in_maps)
            break
        except Exception:
            if attempt == 2:
                raise
    bv = np.asarray(b, dtype=np.float32)[2 * D : 3 * D]  # v bias, host-applied
    out = np.empty((B, S, D), dtype=np.float32)
    for c in range(N_CORES):
        bb, h = c // 2, c % 2
        o_t = res.results[c]["out_t"]  # [D, HALF]
        c3 = res.results[c]["sums_c3"].astype(np.float32)  # [8, 128, QB]
        sums = c3.sum(axis=1).reshape(2, NQB * QB).sum(axis=0)  # [HALF]
        out[bb, h * HALF : (h + 1) * HALF, :] = (o_t / sums).T + bv
    return out


# revision 44
# speedup vs baseline: 1.1401x; 1.1401x over previous
"""Single-head attention (B=4, S=4096, E=2048, d=128) on 8 trn2 cores.

Sharding: core c handles (batch b = c//2, seq half h = c%2). Each core
projects q/k/v for its own 2048-row half; the pair (2b, 2b+1) exchanges
K|V with one 2-core AllGather (~30us doorbell-to-data latency mostly
independent of payload, so a single early doorbell). Softmax over keys
is permutation-invariant, so per-core key order (own-first) is harmless.

Bias algebra: k-bias shifts every key score of a query by a per-query
constant -> softmax-invariant -> dropped. v-bias adds bv to the output
post-normalization -> added on the host. Only the q-bias is applied on
device (folded into the q PSUM evacuation on the DVE).

Measured constraints that shape the schedule: the x load is DMA-bound
(~10MB at ~330GB/s aggregate over 3 queues = all data in at ~40us, and
every projection needs all of it), each engine queue is strict FIFO
(emission order = run order), the PE runs N=512 matmuls at ~216ns
(2.4GHz; the chip sometimes throttles to 2.0GHz run-to-run), and ACT's
~73us exp stream is the attention-phase pace-setter, gated on q (pass
A) and on peer K (pass B). Schedule:
  junk warmup MMs (HAM un-throttle) | x on 3 queues as 32 half-tiles
  load phase: k-FULL + v-FULL matmuls chase the arriving tiles, so the
    K|V AllGather doorbell rings the moment x lands (+evac)
  q in QB-quarters with scores+exp-A chasing each (first exp ~8us
    earlier than half-granularity); own-v PE-transposes ride the
    qb0 scores stream so their DVE copies sit early in the DVE FIFO
  peer V lands pre-transposed via DMA-xbar transposes from cc_out
  PV-A(qb) interleaved per-k-pair with scores-B(qb-1); PV-B0 under
    scores-B3; only PV-B3 + ~2.3us of tree trail the final exp
Denominators: tree levels LA/LB ride the exp stream on the DVE; the
deferred M/N/C3 never stall the PE; the final 128-partition reduction
of the C3 partials happens on the host from a [8, 128, 512] output.
"""

import numpy as np
import ml_dtypes

import concourse.tile as tile
from concourse.masks import make_identity
from concourse import bacc, mybir
from concourse.bass_utils import run_bass_kernel_spmd

N_CORES = 8
B, S, E, D = 4, 4096, 2048, 128
HALF = S // 2  # queries / own keys per core
QB = 512  # query block (PSUM bank width in fp32)
NE = E // 128  # 16 e-chunks
NQB = HALF // QB  # 4 query blocks
SCALE = 1.0 / float(np.sqrt(D))

BF16 = mybir.dt.bfloat16
F32 = mybir.dt.float32
AF = mybir.ActivationFunctionType
GROUPS = [[2 * i, 2 * i + 1] for i in range(N_CORES // 2)]

_CACHE = {}


def _build():
    nc = bacc.Bacc(
        trn_type="TRN2", target_bir_lowering=False, debug=False, num_devices=N_CORES
    )

    x_d = nc.dram_tensor("xt", [E, HALF], BF16, kind="ExternalInput").ap()
    # w packed cg-major: [128, cg(3) * e(16) * 128], cg order (k, q, v)
    w_d = nc.dram_tensor("w", [128, 3 * NE * 128], BF16, kind="ExternalInput").ap()
    bias_d = nc.dram_tensor("bias_q", [D, 1], F32, kind="ExternalInput").ap()
    peer_d = nc.dram_tensor("peer", [1, 1], mybir.dt.uint32, kind="ExternalInput").ap()
    out_d = nc.dram_tensor("out_t", [D, HALF], F32, kind="ExternalOutput").ap()
    # C3-level exp partial sums [pass*4+qb, key-in-chunk, q]; host reduces
    sums_d = nc.dram_tensor("sums_c3", [8, 128, QB], BF16, kind="ExternalOutput").ap()

    with tile.TileContext(nc) as tc:
        with (
            tc.tile_pool(name="xt", bufs=16) as xt_pool,
            tc.tile_pool(name="wsb", bufs=1) as w_pool,
            tc.tile_pool(name="persist", bufs=1) as persist,
            tc.tile_pool(name="exp", bufs=4) as exp_pool,
            tc.tile_pool(name="comb", bufs=2) as comb_pool,
            tc.tile_pool(name="dram", bufs=1, space="DRAM") as dram_pool,
            tc.tile_pool(name="ps", bufs=4, space="PSUM") as ps_pool,
        ):
            # ---- warmup fodder first so the PE can start immediately ----
            junk = persist.tile([128, QB], BF16, tag="junk")
            nc.gpsimd.memset(junk[:], 0.0)
            ps_warm = ps_pool.tile([128, 2 * QB], F32, tag="ps")
            for _ in range(6):
                nc.tensor.matmul(
                    ps_warm[:, 0:QB], lhsT=junk[:, 0:128], rhs=junk[:],
                    start=True, stop=True,
                )
            ident = persist.tile([128, 128], BF16, tag="ident")
            make_identity(nc, ident[:])
            bias_sb = persist.tile([D, 1], F32, tag="bias")
            nc.gpsimd.dma_start(bias_sb[:], bias_d[:])

            # peer slot register (host supplies 1 on even cores, 0 on odd)
            peer_reg = nc.sync.alloc_register("peer_slot")
            nc.sync.reg_load(peer_reg, peer_d[0:1, 0:1])
            peer_val = nc.sync.snap(peer_reg, donate=True, min_val=0, max_val=1)

            # ---- weight / x loads; w interleaved by first use ----
            w_sb = w_pool.tile([128, 3 * NE * 128], BF16, tag="w")
            WG = NE * 128  # one cg = 2048 cols

            def w_ap(cg, e):
                return w_sb[:, cg * WG + e * 128 : cg * WG + (e + 1) * 128]

            def w_load(cg, half, eng):
                lo = cg * WG + half * WG // 2
                eng.dma_start(w_sb[:, lo : lo + WG // 2], w_d[:, lo : lo + WG // 2])

            # k weights on the two fast queues; v/q weights head the scalar
            # queue (idle early); scalar x tiles only from e4 on (its first
            # transfers land late behind the engine preamble)
            w_load(0, 0, nc.sync)
            w_load(0, 1, nc.gpsimd)
            w_load(2, 0, nc.scalar)
            w_load(2, 1, nc.scalar)
            w_load(1, 0, nc.scalar)
            w_load(1, 1, nc.scalar)
            xt = {}
            rr = (nc.sync, nc.gpsimd, nc.scalar)
            for e in range(NE):
                t = xt_pool.tile([128, HALF], BF16, tag="xt")
                xt[e] = t
                for h in range(2):  # half-tiles: finer arrival granularity
                    u = 2 * e + h
                    eng = rr[u % 3] if u >= 4 else (nc.sync, nc.gpsimd)[u % 2]
                    eng.dma_start(
                        t[:, h * 2 * QB : (h + 1) * 2 * QB],
                        x_d[e * 128 : (e + 1) * 128, h * 2 * QB : (h + 1) * 2 * QB],
                    )

            # ---- persistent activations ----
            qT = persist.tile([D, HALF], BF16, tag="qT")
            k_sb = persist.tile([D, S], BF16, tag="k_sb")  # [own kT | peer kT]
            vT_own = persist.tile([D, HALF], BF16, tag="vT_own")
            v_sb = persist.tile([128, (S // 128) * D], BF16, tag="v_sb")
            o_stage = persist.tile([D, HALF], F32, tag="o_stage")

            def k_ap(c):  # kT chunk c (d on partitions); own 0-15, peer 16-31
                return k_sb[:, c * 128 : (c + 1) * 128]

            # ---- load phase: k-FULL + v-FULL chase the arriving x tiles
            # (both gate the combined K|V AllGather; v lags 2 e-chunks so
            # the wv DMA arrives) ----
            ps_ka = ps_pool.tile([128, 2 * QB], F32, tag="ps")
            ps_kb = ps_pool.tile([128, 2 * QB], F32, tag="ps")
            ps_va = ps_pool.tile([128, 2 * QB], F32, tag="ps")
            ps_vb = ps_pool.tile([128, 2 * QB], F32, tag="ps")
            pk = [ps_ka, ps_kb]
            pv = [ps_va, ps_vb]
            LAG = 4  # half-tile units
            for step in range(2 * NE + LAG):
                for cg, pd, u in ((0, pk, step), (2, pv, step - LAG)):
                    if 0 <= u < 2 * NE:
                        e, h = u // 2, u % 2
                        for blk in (2 * h, 2 * h + 1):
                            nc.tensor.matmul(
                                pd[blk // 2][:, (blk % 2) * QB : (blk % 2 + 1) * QB],
                                lhsT=w_ap(cg, e),
                                rhs=xt[e][:, blk * QB : (blk + 1) * QB],
                                start=(e == 0),
                                stop=(e == NE - 1),
                            )
                if step < LAG:  # keep PE busy/warm while DMA ramps
                    for _ in range(2):
                        nc.tensor.matmul(
                            ps_warm[:, 0:QB], lhsT=junk[:, 0:128], rhs=junk[:],
                            start=True, stop=True,
                        )
            for i in range(2):  # ACT is idle until the first exp
                nc.scalar.copy(k_sb[:, i * 2 * QB : (i + 1) * 2 * QB], pk[i][:])
            for i in range(2):
                nc.vector.tensor_copy(
                    vT_own[:, i * 2 * QB : (i + 1) * 2 * QB], pv[i][:]
                )

            # ---- combined K|V exchange (1MB AllGather, one doorbell) ----
            cc_in = dram_pool.tile([D, S], BF16, tag="cc_in")
            cc_out = dram_pool.tile([2, D, S], BF16, tag="cc_out")
            nc.sync.dma_start(cc_in[:, HALF:S], vT_own[:])
            nc.gpsimd.dma_start(cc_in[:, 0:HALF], k_sb[:, 0:HALF])
            nc.gpsimd.collective_compute(
                "AllGather",
                mybir.AluOpType.bypass,
                replica_groups=GROUPS,
                ins=[cc_in.opt()],
                outs=[cc_out.opt()],
            )
            nc.sync.dma_start(k_sb[:, HALF:S], cc_out[peer_val][:, 0:HALF])
            for c in range(16):
                nc.sync.dma_start_transpose(
                    v_sb[:, (16 + c) * D : (17 + c) * D],
                    cc_out[peer_val][:, HALF + c * 128 : HALF + (c + 1) * 128],
                )

            # ---- attention machinery ----
            exp_regions = {}
            comb_regions = {}
            W4, W2, W1 = 4 * QB, 2 * QB, QB

            def scores_kp(qb, p, kp):
                """Scores + exp for one k-pair; tree levels LA/LB ride the
                exp stream (kp3/kp7) so only ~2.3us of tree trails a pass."""
                ex = exp_regions[(qb, p)]
                q_ap = qT[:, qb * QB : (qb + 1) * QB]
                ps = ps_pool.tile([128, 2 * QB], F32, tag="ps")
                for half in range(2):
                    nc.tensor.matmul(
                        ps[:, half * QB : (half + 1) * QB],
                        lhsT=k_ap(16 * p + 2 * kp + half),
                        rhs=q_ap,
                        start=True,
                        stop=True,
                    )
                nc.scalar.activation(
                    ex[:, kp * 2 * QB : (kp + 1) * 2 * QB], ps[:], AF.Exp,
                    scale=SCALE,
                )
                if kp == 3:
                    cb_reg = comb_pool.tile([128, 15 * QB], BF16, tag="comb")
                    comb_regions[(qb, p)] = cb_reg
                    cb = cb_reg
                    nc.vector.tensor_add(
                        cb[:, 0:W4], ex[:, 0:W4], ex[:, W4 : 2 * W4]
                    )
                if kp == 7:
                    cb = comb_regions[(qb, p)]
                    nc.vector.tensor_add(
                        cb[:, W4 : 2 * W4], ex[:, 2 * W4 : 3 * W4],
                        ex[:, 3 * W4 : 4 * W4],
                    )

            def scores_exp(qb, p):
                ex_reg = exp_pool.tile([128, 16 * QB], BF16, tag="exp")
                exp_regions[(qb, p)] = ex_reg
                for kp in range(8):
                    scores_kp(qb, p, kp)

            def pv_kp(qb, p, ps_o, kp):
                ex = exp_regions[(qb, p)]
                for half in range(2):
                    c = 16 * p + 2 * kp + half
                    off = kp * 2 * QB + half * QB
                    nc.tensor.matmul(
                        ps_o[:, 0:QB],
                        lhsT=v_sb[:, c * D : (c + 1) * D],
                        rhs=ex[:, off : off + QB],
                        start=(kp == 0 and half == 0),
                        stop=(kp == 7 and half == 1),
                    )

            def finish_evac(qb, p, ps_o):
                """Output evacuation right after the PV accumulation."""
                exp_regions.pop((qb, p))
                o_sl = o_stage[:, qb * QB : (qb + 1) * QB]
                if p == 0:
                    nc.vector.tensor_copy(o_sl, ps_o[:, 0:QB])
                else:
                    nc.vector.tensor_add(o_sl, o_sl, ps_o[:, 0:QB])
                    nc.sync.dma_start(out_d[:, qb * QB : (qb + 1) * QB], o_sl)

            def finish_sums(qb, p):
                """Deferred M/N/C3 tree; the 128-partition reduction of C3
                happens on the host (no PE/PSUM on this path)."""
                cb = comb_regions.pop((qb, p))
                la, lb = cb[:, 0:W4], cb[:, W4 : 2 * W4]
                m = cb[:, 2 * W4 : 3 * W4]
                n = cb[:, 3 * W4 : 3 * W4 + W2]
                c3 = cb[:, 3 * W4 + W2 : 3 * W4 + W2 + W1]
                nc.vector.tensor_add(m, la, lb)
                nc.vector.tensor_add(n, m[:, 0:W2], m[:, W2 : 2 * W2])
                nc.vector.tensor_add(c3, n[:, 0:W1], n[:, W1 : 2 * W1])
                nc.sync.dma_start(sums_d[4 * p + qb], c3)

            def pv_block(qb, p):
                ps_o = ps_pool.tile([128, 2 * QB], F32, tag="ps")
                for kp in range(8):
                    pv_kp(qb, p, ps_o, kp)
                finish_evac(qb, p, ps_o)

            def project_q(qb):
                """One QB-quarter of the q projection: the first exp only
                needs qb0, so quarters start the ACT stream ~8us earlier."""
                ps = ps_pool.tile([128, 2 * QB], F32, tag="ps")
                for e in range(NE):
                    nc.tensor.matmul(
                        ps[:, 0:QB],
                        lhsT=w_ap(1, e),
                        rhs=xt[e][:, qb * QB : (qb + 1) * QB],
                        start=(e == 0),
                        stop=(e == NE - 1),
                    )
                # evac on ACT: it slots into the exp stream exactly where
                # the dependent scores need it, instead of queuing on the
                # congested DVE FIFO (which made ACT idle 3-4us per block)
                nc.scalar.activation(
                    qT[:, qb * QB : (qb + 1) * QB], ps[:, 0:QB], AF.Identity,
                    bias=bias_sb[:],
                )

            # ---- pass A emission: q halves, exp stream chasing each ----
            def v_transpose(c):
                # bf16 [128, 2048] = same 4KB/partition as the fp32 tiles
                ps_t = ps_pool.tile([128, 4 * QB], BF16, tag="ps")
                nc.tensor.transpose(
                    ps_t[:, 0:128], vT_own[:, c * 128 : (c + 1) * 128], ident[:]
                )
                nc.vector.tensor_copy(v_sb[:, c * D : (c + 1) * D], ps_t[:, 0:128])

            def scores_exp_with_transposes(qb, p):
                """qb0/qb1-A variant: the 16 own-v PE transposes and their
                DVE copies ride the scores/exp stream (2 per k-pair) so the
                copies sit early in the DVE FIFO."""
                ex_reg = exp_pool.tile([128, 16 * QB], BF16, tag="exp")
                exp_regions[(qb, p)] = ex_reg
                for kp in range(8):
                    scores_kp(qb, p, kp)
                    if qb == 0:
                        v_transpose(2 * kp)
                        v_transpose(2 * kp + 1)

            project_q(0)
            scores_exp_with_transposes(0, 0)
            project_q(1)
            scores_exp(1, 0)
            project_q(2)
            scores_exp(2, 0)
            project_q(3)

            # ---- stagger: PV-A(qb) interleaved per-kp with scores-B(qb-1)
            # so ACT streams exp continuously into pass B; sums deferred ----
            pv_block(0, 0)
            scores_exp(3, 0)  # uses the region freed by pv A0
            finish_sums(0, 0)
            for qb_pv, qb_sc in ((1, 0), (2, 1), (3, 2)):
                ex_reg = exp_pool.tile([128, 16 * QB], BF16, tag="exp")
                exp_regions[(qb_sc, 1)] = ex_reg
                ps_o = ps_pool.tile([128, 2 * QB], F32, tag="ps")
                for kp in range(8):
                    pv_kp(qb_pv, 0, ps_o, kp)
                    scores_kp(qb_sc, 1, kp)
                finish_evac(qb_pv, 0, ps_o)
                finish_sums(qb_pv, 0)
            # PV-B0 rides under scores/exp-B3; B1/B2 run during exp-B3;
            # only PV-B3 + ~2.3us of tree trail the final exp
            ex_reg = exp_pool.tile([128, 16 * QB], BF16, tag="exp")
            exp_regions[(3, 1)] = ex_reg
            ps_o0 = ps_pool.tile([128, 2 * QB], F32, tag="ps")
            for kp in range(8):
                pv_kp(0, 1, ps_o0, kp)
                scores_kp(3, 1, kp)
            finish_evac(0, 1, ps_o0)
            for qb in range(1, NQB):
                pv_block(qb, 1)
                finish_sums(qb - 1, 1)
            finish_sums(3, 1)

    nc.compile()
    return nc


def _prep_inputs(x, W, b):
    """Host-side sharding prep: cast bf16, transpose to xT, pack w cg-major."""
    b_f = np.asarray(b, dtype=np.float32)
    bias_q = np.ascontiguousarray(b_f[0:D].reshape(D, 1))  # q bias column
    # W [E, 3D] -> [128p, cg(3), e(16), 128] with cg order (k, q, v)
    w4 = np.asarray(W).astype(ml_dtypes.bfloat16).reshape(NE, 128, 3, D)
    w_bf = np.ascontiguousarray(
        w4.transpose(1, 2, 0, 3)[:, [1, 0, 2], :, :].reshape(128, 3 * NE * D)
    )
    in_maps = []
    for bb in range(B):
        xt_full = np.ascontiguousarray(
            np.asarray(x[bb]).astype(ml_dtypes.bfloat16).T
        )  # [E, S]
        for h in range(2):
            xc = np.ascontiguousarray(xt_full[:, h * HALF : (h + 1) * HALF])
            peer = np.array([[1 - h]], dtype=np.uint32)
            in_maps.append(
                {"xt": xc, "w": w_bf, "bias_q": bias_q, "peer": peer}
            )
    return in_maps


def _run(in_maps, trace=False, trace_kwargs=None):
    if "nc" not in _CACHE:
        _CACHE["nc"] = _build()
    return run_bass_kernel_spmd(
        _CACHE["nc"],
        in_maps,
        list(range(N_CORES)),
        trace=trace,
        **(trace_kwargs or {}),
    )


def kernel(x, W, b):
    in_maps = _prep_inputs(x, W, b)
    res = None
    for attempt in range(3):
        try:
            res = _run(in_maps)
            break
        except Exception:
            if attempt == 2:
                raise
    bv = np.asarray(b, dtype=np.float32)[2 * D : 3 * D]  # v bias, host-applied
    out = np.empty((B, S, D), dtype=np.float32)
    for c in range(N_CORES):
        bb, h = c // 2, c % 2
        o_t = res.results[c]["out_t"]  # [D, HALF]
        c3 = res.results[c]["sums_c3"].astype(np.float32)  # [8, 128, QB]
        sums = c3.sum(axis=1).reshape(2, NQB * QB).sum(axis=0)  # [HALF]
        out[bb, h * HALF : (h + 1) * HALF, :] = (o_t / sums).T + bv
    return out
